# revision 6
# baseline (speedup 1.0000x reference)
"""Trainium2 Bass kernel for NgramRF GNN message passing (8-core SPMD).

Strategy:
- Nodes are re-laid-out on host: sorted-by-graph node order, padded so each
  of the 8 cores owns exactly 12800 nodes covering exactly 512 graphs
  (102400 padded node slots total). Pad nodes have no edges and are never
  pooled; BN statistics divide by the true node count.
- Message passing m[v] = sum_{(u,v)} h[u] is computed per core for its dst
  shard: edges are gathered 128 at a time with dma_gather (int16 indices ->
  4 source buckets of 25600 rows), then scatter-added via TensorE matmul
  with a one-hot dst matrix into PSUM [128 feat x 512 dst] windows.
- h lives replicated in each core's DRAM ([102400, 128]); per round each
  core produces its shard's new h and an AllGather rebuilds the table.
- BN is training-mode over all nodes: per-core partial sums + tiny
  AllReduce, then a fused scale/bias/ReLU on the Activation engine.
- Pooling (every 3rd round) taps the node-major transpose stream in SBUF:
  one matmul per 128-node chunk into a [128 feat x 512 graph] PSUM.
"""
import os
import numpy as np

P = 128
N_NODES = 100000
N_EDGES = 1600000
N_GRAPHS = 4096
IN_FEAT = 74
HID = 128
N_CONV = 3
NGRAM = 6
N_ROUNDS = int(os.environ.get("KROUNDS", N_CONV * NGRAM))
BN_EPS = 1e-5

N_CORES = 8
SH = 12800              # nodes per core (padded)
NPAD = SH * N_CORES     # 102400
BUCKET = 25600          # gather table bucket rows (int16-addressable)
N_BK = 4                # buckets
N_WIN = 25              # dst windows of 512 per core
WIN = 512
G_SH = 512              # graphs per core
N_CHUNK = SH // P       # 100 transpose chunks per core
CALL_TILES = 16         # tiles per dma_gather call (2048 idxs)
OH_GROUP = 4            # onehot tiles built per DVE op

PAD_DST = 600.0         # sentinel dst_local -> all-zero onehot column
PAD_G = 600.0           # sentinel graph_local


def _pack_idxs(arr):
    """[N] int16 -> [128, N//16] wrapped in 16 partitions, replicated x8."""
    n = arr.shape[0]
    w = arr.reshape(n // 16, 16).T.astype(np.int16)
    return np.tile(w, (8, 1))


def _prep(features, src, dst, graph_ids):
    """Host-side: node remap (graph-aligned shards), edge tiling, index arrays."""
    gid = np.asarray(graph_ids)
    src = np.asarray(src).astype(np.int64)
    dst = np.asarray(dst).astype(np.int64)

    # node remap: nodes already sorted by graph id; insert pads so that each
    # core's 12800 slots hold the nodes of graphs [512c, 512(c+1)).
    g_start = np.searchsorted(gid, np.arange(0, N_GRAPHS + 1, G_SH))  # [9]
    remap = np.full(N_NODES, -1, dtype=np.int64)
    new_gid = np.full(NPAD, -1, dtype=np.int64)
    for c in range(N_CORES):
        lo, hi = g_start[c], g_start[c + 1]
        cnt = hi - lo
        assert cnt <= SH, f"core {c} has {cnt} nodes > {SH}"
        remap[lo:hi] = SH * c + np.arange(cnt)
        new_gid[SH * c : SH * c + cnt] = gid[lo:hi]
    src_n = remap[src]
    dst_n = remap[dst]

    # per-core padded features
    feat_shards = []
    f = np.asarray(features, dtype=np.float32)
    for c in range(N_CORES):
        lo, hi = g_start[c], g_start[c + 1]
        sh = np.zeros((SH, IN_FEAT), dtype=np.float32)
        sh[: hi - lo] = f[lo:hi]
        feat_shards.append(sh)

    # edge grouping: core = dst//SH, window b = (dst%SH)//WIN, bucket k = src//BUCKET
    core_e = dst_n // SH
    b_e = (dst_n % SH) // WIN
    k_e = src_n // BUCKET
    # counts per (c, b, k)
    key = (core_e * N_WIN + b_e) * N_BK + k_e
    counts = np.bincount(key, minlength=N_CORES * N_WIN * N_BK).reshape(
        N_CORES, N_WIN, N_BK
    )
    T = np.maximum(1, -(-counts.max(axis=0) // P))  # [N_WIN, N_BK] tiles per group

    order = np.argsort(key, kind="stable")
    src_s, dst_s, key_s = src_n[order], dst_n[order], key[order]
    # start offset of each (c,b,k) run in the sorted arrays
    starts = np.zeros(N_CORES * N_WIN * N_BK + 1, dtype=np.int64)
    np.cumsum(np.bincount(key_s, minlength=N_CORES * N_WIN * N_BK), out=starts[1:])

    NT = int(T.sum())  # tiles per pass-major stream per core
    gidx_shards, dstloc_shards = [], []
    for c in range(N_CORES):
        idx_stream = np.zeros(NT * P, dtype=np.int16)
        dl_stream = np.full(NT * P, PAD_DST, dtype=np.float32)
        pos = 0
        for k in range(N_BK):
            for b in range(N_WIN):
                gi = (c * N_WIN + b) * N_BK + k
                s, e = starts[gi], starts[gi + 1]
                n = e - s
                cap = T[b, k] * P
                assert n <= cap
                idx_stream[pos : pos + n] = (src_s[s:e] - BUCKET * k).astype(np.int16)
                dl_stream[pos : pos + n] = (dst_s[s:e] - SH * c - WIN * b).astype(
                    np.float32
                )
                pos += cap
        assert pos == NT * P
        gidx_shards.append(_pack_idxs(idx_stream))
        dstloc_shards.append(
            dl_stream.reshape(NT, P).T.astype(np.float32)
        )  # [128, NT]

    # graph-local ids per transpose chunk: [128, N_CHUNK] per core
    glocal_shards = []
    for c in range(N_CORES):
        gl = new_gid[SH * c : SH * (c + 1)].astype(np.float32)
        gl = np.where(gl < 0, PAD_G, gl - G_SH * c)
        glocal_shards.append(gl.reshape(N_CHUNK, P).T.copy())  # [128, 100]

    # per-pass tile counts -> dma_gather call plan (static, same all cores)
    pass_tiles = [int(T[:, k].sum()) for k in range(N_BK)]
    return dict(
        T=T,
        NT=NT,
        pass_tiles=pass_tiles,
        feat=feat_shards,
        gidx=gidx_shards,
        dstloc=dstloc_shards,
        glocal=glocal_shards,
        g_start=g_start,
    )


def _build(T, NT, pass_tiles, ngram_w):
    import concourse.bass as bass
    import concourse.bacc as bacc
    import concourse.mybir as mybir
    import concourse.tile as tile
    from concourse.masks import make_identity

    dt = mybir.dt
    AF = mybir.ActivationFunctionType
    OP = mybir.AluOpType
    RG = [list(range(N_CORES))]

    nc = bacc.Bacc("TRN2", target_bir_lowering=False, debug=False,
                   num_devices=N_CORES)

    # ---- I/O ----
    feat_in = nc.dram_tensor("feat", [SH, IN_FEAT], dt.float32, kind="ExternalInput")
    gidx_in = nc.dram_tensor("gidx", [P, NT * 8], dt.int16, kind="ExternalInput")
    dstloc_in = nc.dram_tensor("dstloc", [P, NT], dt.float32, kind="ExternalInput")
    glocal_in = nc.dram_tensor("glocal", [P, N_CHUNK], dt.float32, kind="ExternalInput")
    w_in_in = nc.dram_tensor("w_in", [IN_FEAT, HID], dt.float32, kind="ExternalInput")
    conv_in = nc.dram_tensor("conv_w", [N_CONV, HID, HID], dt.float32, kind="ExternalInput")
    gamma_in = nc.dram_tensor("gamma_t", [P, N_CONV], dt.float32, kind="ExternalInput")
    beta_in = nc.dram_tensor("beta_t", [P, N_CONV], dt.float32, kind="ExternalInput")
    w1_in = nc.dram_tensor("w1", [HID, HID], dt.float32, kind="ExternalInput")
    b1_in = nc.dram_tensor("b1_t", [P, 1], dt.float32, kind="ExternalInput")
    w2_in = nc.dram_tensor("w2", [HID, 1], dt.float32, kind="ExternalInput")
    b2_in = nc.dram_tensor("b2_t", [1, 1], dt.float32, kind="ExternalInput")
    out_t = nc.dram_tensor("out", [1, G_SH], dt.float32, kind="ExternalOutput")

    # ---- internal DRAM ----
    h_tbl = nc.dram_tensor("h_tbl", [NPAD, HID], dt.float32, kind="Internal",
                           addr_space="Shared")
    ag_in = nc.dram_tensor("ag_in", [SH, HID], dt.float32, kind="Internal")
    ar_in = nc.dram_tensor("ar_in", [P, 2], dt.float32, kind="Internal")
    ar_out = nc.dram_tensor("ar_out", [P, 2], dt.float32, kind="Internal",
                            addr_space="Shared")

    inv_n = 1.0 / N_NODES

    with tile.TileContext(nc) as tc:
        with (
            tc.tile_pool(name="const", bufs=1) as constp,
            tc.tile_pool(name="gath", bufs=3) as gathp,
            tc.tile_pool(name="oh", bufs=2) as ohp,
            tc.tile_pool(name="mpsum", bufs=2, space="PSUM") as mpsum,
            tc.tile_pool(name="bpsum", bufs=2, space="PSUM") as bpsum,
            tc.tile_pool(name="tpsum", bufs=2, space="PSUM") as tpsum,
            tc.tile_pool(name="ppsum", bufs=1, space="PSUM") as ppsum,
            tc.tile_pool(name="hn", bufs=4) as hnp,
            tc.tile_pool(name="small", bufs=4) as smallp,
        ):
            # ---- resident constants ----
            idx_sb = constp.tile([P, NT * 8], dt.int16)
            nc.sync.dma_start(out=idx_sb[:], in_=gidx_in[:, :])
            dstloc_sb = constp.tile([P, NT], dt.float32)
            nc.sync.dma_start(out=dstloc_sb[:], in_=dstloc_in[:, :])
            glocal_sb = constp.tile([P, N_CHUNK], dt.float32)
            nc.sync.dma_start(out=glocal_sb[:], in_=glocal_in[:, :])
            w_in_sb = constp.tile([IN_FEAT, HID], dt.float32)
            nc.sync.dma_start(out=w_in_sb[:], in_=w_in_in[:, :])
            conv_sb = [
                constp.tile([HID, HID], dt.float32, tag=f"cw{j}", name=f"cw{j}")
                for j in range(N_CONV)
            ]
            for j in range(N_CONV):
                nc.sync.dma_start(out=conv_sb[j][:], in_=conv_in[j, :, :])
            gamma_sb = constp.tile([P, N_CONV], dt.float32)
            nc.sync.dma_start(out=gamma_sb[:], in_=gamma_in[:, :])
            beta_sb = constp.tile([P, N_CONV], dt.float32)
            nc.sync.dma_start(out=beta_sb[:], in_=beta_in[:, :])
            w1_sb = constp.tile([HID, HID], dt.float32)
            nc.sync.dma_start(out=w1_sb[:], in_=w1_in[:, :])
            b1_sb = constp.tile([P, 1], dt.float32)
            nc.sync.dma_start(out=b1_sb[:], in_=b1_in[:, :])
            w2_sb = constp.tile([HID, 1], dt.float32)
            nc.sync.dma_start(out=w2_sb[:], in_=w2_in[:, :])
            b2_sb = constp.tile([1, 1], dt.float32)
            nc.sync.dma_start(out=b2_sb[:], in_=b2_in[:, :])

            ident = constp.tile([P, P], dt.float32)
            make_identity(nc, ident[:])
            iota_i = constp.tile([P, WIN], dt.int32)
            nc.gpsimd.iota(iota_i[:], pattern=[[1, WIN]], base=0, channel_multiplier=0)
            iota_f = constp.tile([P, WIN], dt.float32)
            nc.vector.tensor_copy(out=iota_f[:], in_=iota_i[:])
            zero_sb = constp.tile([P, 1], dt.float32)
            nc.vector.memset(zero_sb[:], 0.0)

            m_sb = constp.tile([P, SH], dt.float32)       # m then y then h_T
            comb_sb = constp.tile([P, G_SH], dt.float32)  # pooled combination

            # ================= h0 = features @ W_in =================
            for j in range(N_CHUNK):
                fch = hnp.tile([P, IN_FEAT], dt.float32, tag="fch")
                nc.sync.dma_start(out=fch[:], in_=feat_in[j * P : (j + 1) * P, :])
                ftp = tpsum.tile([IN_FEAT, P], dt.float32, tag="tp", name="ftp")
                nc.tensor.transpose(out=ftp[:], in_=fch[:], identity=ident[:])
                ft = hnp.tile([IN_FEAT, P], dt.float32, tag="ft")
                nc.vector.tensor_copy(out=ft[:], in_=ftp[:])
                h0p = tpsum.tile([P, HID], dt.float32, tag="tp", name="h0p")
                nc.tensor.matmul(out=h0p[:], lhsT=ft[:], rhs=w_in_sb[:],
                                 start=True, stop=True)
                h0t = hnp.tile([P, HID], dt.float32, tag="h0t")
                nc.vector.tensor_copy(out=h0t[:], in_=h0p[:])
                nc.sync.dma_start(out=ag_in[j * P : (j + 1) * P, :], in_=h0t[:])
            nc.gpsimd.collective_compute(
                "AllGather", OP.bypass, replica_groups=RG,
                ins=[ag_in[:, :]], outs=[h_tbl[:, :]],
            )

            # ================= 18 message-passing rounds =================
            for r in range(N_ROUNDS):
                j = r % N_CONV
                is_pool = (r % N_CONV) == N_CONV - 1
                pool_i = r // N_CONV
                last = r == N_ROUNDS - 1

                # ---- phase A: m_T = scatter-add of gathered h ----
                tbase = 0  # tile index in the stream
                for k in range(N_BK):
                    # gather calls for this pass
                    ptiles = pass_tiles[k]
                    gts = []  # (tile, offset_in_call) per tile of this pass
                    done = 0
                    while done < ptiles:
                        n_t = min(CALL_TILES, ptiles - done)
                        g = gathp.tile([P, n_t, HID], dt.float32, tag="g")
                        col = (tbase + done) * 8
                        nc.gpsimd.dma_gather(
                            g[:], h_tbl[BUCKET * k : BUCKET * (k + 1), :],
                            idx_sb[:, col : col + n_t * 8],
                            n_t * P, n_t * P, HID, single_packet=False,
                        )
                        for t in range(n_t):
                            gts.append((g, t))
                        done += n_t
                    # onehot + matmul per (b) group
                    ti = 0
                    for b in range(N_WIN):
                        tb = int(T[b, k])
                        ps = mpsum.tile([HID, WIN], dt.float32, tag="mps")
                        t0 = 0
                        while t0 < tb:
                            ng = min(OH_GROUP, tb - t0)
                            oh = ohp.tile([P, ng, WIN], dt.float32, tag="oh")
                            c0 = tbase + ti + t0
                            nc.vector.tensor_tensor(
                                out=oh[:],
                                in0=dstloc_sb[:, c0 : c0 + ng, None].to_broadcast(
                                    [P, ng, WIN]
                                ),
                                in1=iota_f[:, None, :].to_broadcast([P, ng, WIN]),
                                op=OP.is_equal,
                            )
                            for t in range(ng):
                                gt, off = gts[ti + t0 + t]
                                nc.tensor.matmul(
                                    out=ps[:],
                                    lhsT=gt[:, off, :],
                                    rhs=oh[:, t, :],
                                    start=(t0 + t == 0),
                                    stop=(t0 + t == tb - 1),
                                )
                            t0 += ng
                        dst_sl = m_sb[:, WIN * b : WIN * (b + 1)]
                        if k == 0:
                            nc.vector.tensor_copy(out=dst_sl, in_=ps[:])
                        else:
                            nc.vector.tensor_add(out=dst_sl, in0=dst_sl, in1=ps[:])
                        ti += tb
                    tbase += ptiles

                # ---- phase B: y = m @ conv_w[j], BN stats ----
                sum_acc = smallp.tile([P, 1], dt.float32, tag="sum")
                sq_acc = smallp.tile([P, 1], dt.float32, tag="sq")
                for w in range(N_WIN):
                    ps = bpsum.tile([HID, WIN], dt.float32, tag="bps")
                    nc.tensor.matmul(out=ps[:], lhsT=conv_sb[j][:],
                                     rhs=m_sb[:, WIN * w : WIN * (w + 1)],
                                     start=True, stop=True)
                    csum = smallp.tile([P, 1], dt.float32, tag="csum")
                    nc.vector.tensor_reduce(out=csum[:], in_=ps[:], op=OP.add,
                                            axis=mybir.AxisListType.X)
                    sq_scr = hnp.tile([P, WIN], dt.float32, tag="sqscr")
                    csq = smallp.tile([P, 1], dt.float32, tag="csq")
                    nc.scalar.activation(out=sq_scr[:], in_=ps[:], func=AF.Square,
                                         bias=zero_sb[:], accum_out=csq[:])
                    if w == 0:
                        nc.vector.tensor_copy(out=sum_acc[:], in_=csum[:])
                        nc.vector.tensor_copy(out=sq_acc[:], in_=csq[:])
                    else:
                        nc.vector.tensor_add(out=sum_acc[:], in0=sum_acc[:], in1=csum[:])
                        nc.vector.tensor_add(out=sq_acc[:], in0=sq_acc[:], in1=csq[:])
                    nc.vector.tensor_copy(out=m_sb[:, WIN * w : WIN * (w + 1)], in_=ps[:])

                # ---- BN stats allreduce + scale/bias ----
                stat_sb = smallp.tile([P, 2], dt.float32, tag="stat")
                nc.vector.tensor_copy(out=stat_sb[:, 0:1], in_=sum_acc[:])
                nc.vector.tensor_copy(out=stat_sb[:, 1:2], in_=sq_acc[:])
                nc.sync.dma_start(out=ar_in[:, :], in_=stat_sb[:])
                nc.gpsimd.collective_compute(
                    "AllReduce", OP.add, replica_groups=RG,
                    ins=[ar_in[:, :]], outs=[ar_out[:, :]],
                )
                stg = smallp.tile([P, 2], dt.float32, tag="stg")
                nc.sync.dma_start(out=stg[:], in_=ar_out[:, :])
                mean = smallp.tile([P, 1], dt.float32, tag="mean")
                nc.vector.tensor_scalar_mul(out=mean[:], in0=stg[:, 0:1], scalar1=inv_n)
                ex2 = smallp.tile([P, 1], dt.float32, tag="ex2")
                nc.vector.tensor_scalar_mul(out=ex2[:], in0=stg[:, 1:2], scalar1=inv_n)
                var = smallp.tile([P, 1], dt.float32, tag="var")
                nc.vector.tensor_tensor(out=var[:], in0=mean[:], in1=mean[:], op=OP.mult)
                nc.vector.tensor_tensor(out=var[:], in0=ex2[:], in1=var[:], op=OP.subtract)
                sd = smallp.tile([P, 1], dt.float32, tag="sd")
                nc.vector.tensor_scalar_add(out=var[:], in0=var[:], scalar1=BN_EPS)
                nc.scalar.activation(out=sd[:], in_=var[:], func=AF.Sqrt,
                                     bias=zero_sb[:])
                rs = smallp.tile([P, 1], dt.float32, tag="rs")
                nc.vector.reciprocal(out=rs[:], in_=sd[:])
                s_t = smallp.tile([P, 1], dt.float32, tag="s_t")
                nc.vector.tensor_tensor(out=s_t[:], in0=rs[:], in1=gamma_sb[:, j : j + 1], op=OP.mult)
                b_t = smallp.tile([P, 1], dt.float32, tag="b_t")
                nc.vector.tensor_tensor(out=b_t[:], in0=mean[:], in1=s_t[:], op=OP.mult)
                nc.vector.tensor_tensor(out=b_t[:], in0=beta_sb[:, j : j + 1], in1=b_t[:], op=OP.subtract)

                # ---- normalize + relu in place (h_T now in m_sb) ----
                for w in range(N_WIN):
                    sl = m_sb[:, WIN * w : WIN * (w + 1)]
                    nc.scalar.activation(out=sl, in_=sl, func=AF.Relu,
                                         bias=b_t[:], scale=s_t[:])

                # ---- transpose to node-major; pool taps; AG input ----
                if is_pool:
                    pps = ppsum.tile([HID, G_SH], dt.float32, tag="pps")
                for j100 in range(N_CHUNK):
                    tp = tpsum.tile([P, P], dt.float32, tag="tp")
                    nc.tensor.transpose(
                        out=tp[:], in_=m_sb[:, P * j100 : P * (j100 + 1)],
                        identity=ident[:],
                    )
                    hn = hnp.tile([P, HID], dt.float32, tag="hn")
                    nc.vector.tensor_copy(out=hn[:], in_=tp[:])
                    if is_pool:
                        ohg = ohp.tile([P, WIN], dt.float32, tag="ohg")
                        nc.vector.tensor_tensor(
                            out=ohg[:],
                            in0=glocal_sb[:, j100 : j100 + 1].to_broadcast([P, WIN]),
                            in1=iota_f[:],
                            op=OP.is_equal,
                        )
                        nc.tensor.matmul(out=pps[:], lhsT=hn[:], rhs=ohg[:],
                                         start=(j100 == 0), stop=(j100 == N_CHUNK - 1))
                    if not last:
                        nc.sync.dma_start(
                            out=ag_in[P * j100 : P * (j100 + 1), :], in_=hn[:]
                        )
                if is_pool:
                    wgt = float(ngram_w[pool_i])
                    if pool_i == 0:
                        nc.scalar.activation(out=comb_sb[:], in_=pps[:],
                                             func=AF.Copy, scale=wgt)
                    else:
                        ptmp = hnp.tile([P, G_SH], dt.float32, tag="ptmp")
                        nc.scalar.activation(out=ptmp[:], in_=pps[:],
                                             func=AF.Copy, scale=wgt)
                        nc.vector.tensor_add(out=comb_sb[:], in0=comb_sb[:], in1=ptmp[:])
                if not last:
                    nc.gpsimd.collective_compute(
                        "AllGather", OP.bypass, replica_groups=RG,
                        ins=[ag_in[:, :]], outs=[h_tbl[:, :]],
                    )

            # ================= head =================
            z1p = bpsum.tile([HID, G_SH], dt.float32, tag="bps", name="z1p")
            nc.tensor.matmul(out=z1p[:], lhsT=w1_sb[:], rhs=comb_sb[:],
                             start=True, stop=True)
            z1 = hnp.tile([P, G_SH], dt.float32, tag="z1")
            nc.scalar.activation(out=z1[:], in_=z1p[:], func=AF.Lrelu,
                                 bias=b1_sb[:], alpha=0.01)
            z2p = bpsum.tile([1, G_SH], dt.float32, tag="bps", name="z2p")
            nc.tensor.matmul(out=z2p[:], lhsT=w2_sb[:], rhs=z1[:],
                             start=True, stop=True)
            z2 = smallp.tile([1, G_SH], dt.float32, tag="z2")
            nc.scalar.activation(out=z2[:], in_=z2p[:], func=AF.Sigmoid,
                                 bias=b2_sb[:1, :])
            nc.sync.dma_start(out=out_t[:, :], in_=z2[:])

    nc.compile()
    return nc


_CACHE = {}


def kernel(features, W_in, conv_w, bn_gamma, bn_beta, ngram_weights,
           W1, b1, W2, b2, src, dst, graph_ids):
    from concourse import bass_utils

    prep = _prep(features, src, dst, graph_ids)

    ngw = np.asarray(ngram_weights, dtype=np.float64)
    e = np.exp(ngw - ngw.max())
    ngram_w = (e / e.sum()).astype(np.float64)

    key = (prep["NT"], tuple(prep["pass_tiles"]), tuple(np.asarray(ngram_w).tolist()))
    if key not in _CACHE:
        _CACHE[key] = _build(prep["T"], prep["NT"], prep["pass_tiles"], ngram_w)
    nc = _CACHE[key]

    gamma_t = np.asarray(bn_gamma, dtype=np.float32).T.copy()  # [128, 3]
    beta_t = np.asarray(bn_beta, dtype=np.float32).T.copy()
    b1_t = np.asarray(b1, dtype=np.float32).reshape(P, 1)
    b2_t = np.asarray(b2, dtype=np.float32).reshape(1, 1)

    in_maps = []
    for c in range(N_CORES):
        in_maps.append({
            "feat": prep["feat"][c],
            "gidx": prep["gidx"][c],
            "dstloc": prep["dstloc"][c],
            "glocal": prep["glocal"][c],
            "w_in": np.asarray(W_in, dtype=np.float32),
            "conv_w": np.asarray(conv_w, dtype=np.float32),
            "gamma_t": gamma_t,
            "beta_t": beta_t,
            "w1": np.asarray(W1, dtype=np.float32),
            "b1_t": b1_t,
            "w2": np.asarray(W2, dtype=np.float32),
            "b2_t": b2_t,
        })

    trace = bool(int(os.environ.get("KTRACE", "0")))
    if trace:
        try:
            import sys, types
            if "antenv.axon_hooks" not in sys.modules:
                mod = types.ModuleType("antenv.axon_hooks")
                _h = [None]
                mod.set_axon_ntff_profile_hook = lambda h: _h.__setitem__(0, h)
                mod.get_axon_ntff_profile_hook = lambda: _h[0]
                sys.modules["antenv.axon_hooks"] = mod
                import antenv
                antenv.axon_hooks = mod
            from antenv.axon_hooks import get_axon_ntff_profile_hook, set_axon_ntff_profile_hook
            if get_axon_ntff_profile_hook() is None:
                from trn_agent_boot.trn_boot import _ntff_profile_via_ctypes
                set_axon_ntff_profile_hook(
                    _ntff_profile_via_ctypes("/opt/axon/libaxon_pjrt.so"))
        except Exception:
            trace = False
    res = bass_utils.run_bass_kernel_spmd(nc, in_maps, core_ids=list(range(N_CORES)),
                                          trace=trace)
    if trace and res.exec_time_ns is not None:
        print(f"HW exec time: {res.exec_time_ns} ns")
    out = np.concatenate([res.results[c]["out"][0] for c in range(N_CORES)])
    return out.reshape(N_GRAPHS, 1).astype(np.float32)


# revision 7
# speedup vs baseline: 5.7844x; 5.7844x over previous
"""Trainium2 Bass kernel for NgramRF GNN message passing (8-core SPMD).

Strategy:
- Nodes are re-laid-out on host: sorted-by-graph node order, padded so each
  of the 8 cores owns exactly 12800 nodes covering exactly 512 graphs
  (102400 padded node slots total). Pad nodes have no edges and are never
  pooled; BN statistics divide by the true node count.
- Message passing m[v] = sum_{(u,v)} h[u] is computed per core for its dst
  shard: edges are gathered 128 at a time with dma_gather (int16 indices ->
  4 source buckets of 25600 rows), then scatter-added via TensorE matmul
  with a one-hot dst matrix into PSUM [128 feat x 512 dst] windows.
- h lives replicated in each core's DRAM ([102400, 128]); per round each
  core produces its shard's new h and an AllGather rebuilds the table.
- BN is training-mode over all nodes: per-core partial sums + tiny
  AllReduce, then a fused scale/bias/ReLU on the Activation engine.
- Pooling (every 3rd round) taps the node-major transpose stream in SBUF:
  one matmul per 128-node chunk into a [128 feat x 512 graph] PSUM.
"""
import os
import numpy as np

P = 128
N_NODES = 100000
N_EDGES = 1600000
N_GRAPHS = 4096
IN_FEAT = 74
HID = 128
N_CONV = 3
NGRAM = 6
N_ROUNDS = int(os.environ.get("KROUNDS", N_CONV * NGRAM))
BN_EPS = 1e-5

N_CORES = 8
SH = 12800              # nodes per core (padded)
NPAD = SH * N_CORES     # 102400
BUCKET = 25600          # gather table bucket rows (int16-addressable)
N_BK = 4                # buckets
N_WIN = 25              # dst windows of 512 per core
WIN = 512
G_SH = 512              # graphs per core
N_CHUNK = SH // P       # 100 transpose chunks per core
CALL_TILES = 32         # tiles per dma_gather call (4096 idxs)
OH_GROUP = 4            # onehot tiles built per DVE op

PAD_DST = 600.0         # sentinel dst_local -> all-zero onehot column
PAD_G = 600.0           # sentinel graph_local


def _pack_idxs(arr):
    """[N] int16 -> [128, N//16] wrapped in 16 partitions, replicated x8."""
    n = arr.shape[0]
    w = arr.reshape(n // 16, 16).T.astype(np.int16)
    return np.tile(w, (8, 1))


def _prep(features, src, dst, graph_ids):
    """Host-side: node remap (graph-aligned shards), edge tiling, index arrays."""
    gid = np.asarray(graph_ids)
    src = np.asarray(src).astype(np.int64)
    dst = np.asarray(dst).astype(np.int64)

    # node remap: nodes already sorted by graph id; insert pads so that each
    # core's 12800 slots hold the nodes of graphs [512c, 512(c+1)).
    g_start = np.searchsorted(gid, np.arange(0, N_GRAPHS + 1, G_SH))  # [9]
    remap = np.full(N_NODES, -1, dtype=np.int64)
    new_gid = np.full(NPAD, -1, dtype=np.int64)
    for c in range(N_CORES):
        lo, hi = g_start[c], g_start[c + 1]
        cnt = hi - lo
        assert cnt <= SH, f"core {c} has {cnt} nodes > {SH}"
        remap[lo:hi] = SH * c + np.arange(cnt)
        new_gid[SH * c : SH * c + cnt] = gid[lo:hi]
    src_n = remap[src]
    dst_n = remap[dst]

    # per-core padded features
    feat_shards = []
    f = np.asarray(features, dtype=np.float32)
    for c in range(N_CORES):
        lo, hi = g_start[c], g_start[c + 1]
        sh = np.zeros((SH, IN_FEAT), dtype=np.float32)
        sh[: hi - lo] = f[lo:hi]
        feat_shards.append(sh)

    # edge grouping: core = dst//SH, window b = (dst%SH)//WIN, bucket k = src//BUCKET
    core_e = dst_n // SH
    b_e = (dst_n % SH) // WIN
    k_e = src_n // BUCKET
    # counts per (c, b, k)
    key = (core_e * N_WIN + b_e) * N_BK + k_e
    counts = np.bincount(key, minlength=N_CORES * N_WIN * N_BK).reshape(
        N_CORES, N_WIN, N_BK
    )
    T = np.maximum(1, -(-counts.max(axis=0) // P))  # [N_WIN, N_BK] tiles per group

    order = np.argsort(key, kind="stable")
    src_s, dst_s, key_s = src_n[order], dst_n[order], key[order]
    # start offset of each (c,b,k) run in the sorted arrays
    starts = np.zeros(N_CORES * N_WIN * N_BK + 1, dtype=np.int64)
    np.cumsum(np.bincount(key_s, minlength=N_CORES * N_WIN * N_BK), out=starts[1:])

    NT = int(T.sum())  # tiles per pass-major stream per core
    gidx_shards, dstloc_shards = [], []
    for c in range(N_CORES):
        idx_stream = np.zeros(NT * P, dtype=np.int16)
        dl_stream = np.full(NT * P, PAD_DST, dtype=np.float32)
        pos = 0
        for k in range(N_BK):
            for b in range(N_WIN):
                gi = (c * N_WIN + b) * N_BK + k
                s, e = starts[gi], starts[gi + 1]
                n = e - s
                cap = T[b, k] * P
                assert n <= cap
                idx_stream[pos : pos + n] = (src_s[s:e] - BUCKET * k).astype(np.int16)
                dl_stream[pos : pos + n] = (dst_s[s:e] - SH * c - WIN * b).astype(
                    np.float32
                )
                pos += cap
        assert pos == NT * P
        gidx_shards.append(_pack_idxs(idx_stream))
        dstloc_shards.append(
            dl_stream.reshape(NT, P).T.astype(np.float32)
        )  # [128, NT]

    # graph-local ids per transpose chunk: [128, N_CHUNK] per core
    glocal_shards = []
    for c in range(N_CORES):
        gl = new_gid[SH * c : SH * (c + 1)].astype(np.float32)
        gl = np.where(gl < 0, PAD_G, gl - G_SH * c)
        glocal_shards.append(gl.reshape(N_CHUNK, P).T.copy())  # [128, 100]

    # per-pass tile counts -> dma_gather call plan (static, same all cores)
    pass_tiles = [int(T[:, k].sum()) for k in range(N_BK)]
    return dict(
        T=T,
        NT=NT,
        pass_tiles=pass_tiles,
        feat=feat_shards,
        gidx=gidx_shards,
        dstloc=dstloc_shards,
        glocal=glocal_shards,
        g_start=g_start,
    )


def _build(T, NT, pass_tiles, ngram_w):
    import concourse.bass as bass
    import concourse.bacc as bacc
    import concourse.mybir as mybir
    import concourse.tile as tile
    from concourse.masks import make_identity

    dt = mybir.dt
    AF = mybir.ActivationFunctionType
    OP = mybir.AluOpType
    RG = [list(range(N_CORES))]

    nc = bacc.Bacc("TRN2", target_bir_lowering=False, debug=False,
                   num_devices=N_CORES)

    # ---- I/O ----
    feat_in = nc.dram_tensor("feat", [SH, IN_FEAT], dt.float32, kind="ExternalInput")
    gidx_in = nc.dram_tensor("gidx", [P, NT * 8], dt.int16, kind="ExternalInput")
    dstloc_in = nc.dram_tensor("dstloc", [P, NT], dt.float32, kind="ExternalInput")
    glocal_in = nc.dram_tensor("glocal", [P, N_CHUNK], dt.float32, kind="ExternalInput")
    w_in_in = nc.dram_tensor("w_in", [IN_FEAT, HID], dt.float32, kind="ExternalInput")
    conv_in = nc.dram_tensor("conv_w", [N_CONV, HID, HID], dt.float32, kind="ExternalInput")
    gamma_in = nc.dram_tensor("gamma_t", [P, N_CONV], dt.float32, kind="ExternalInput")
    beta_in = nc.dram_tensor("beta_t", [P, N_CONV], dt.float32, kind="ExternalInput")
    w1_in = nc.dram_tensor("w1", [HID, HID], dt.float32, kind="ExternalInput")
    b1_in = nc.dram_tensor("b1_t", [P, 1], dt.float32, kind="ExternalInput")
    w2_in = nc.dram_tensor("w2", [HID, 1], dt.float32, kind="ExternalInput")
    b2_in = nc.dram_tensor("b2_t", [1, 1], dt.float32, kind="ExternalInput")
    out_t = nc.dram_tensor("out", [1, G_SH], dt.float32, kind="ExternalOutput")

    # ---- internal DRAM ----
    h_tbl = nc.dram_tensor("h_tbl", [NPAD, HID], dt.float32, kind="Internal",
                           addr_space="Shared")
    ag_in = nc.dram_tensor("ag_in", [SH, HID], dt.float32, kind="Internal")
    ar_in = nc.dram_tensor("ar_in", [P, 2], dt.float32, kind="Internal")
    ar_out = nc.dram_tensor("ar_out", [P, 2], dt.float32, kind="Internal",
                            addr_space="Shared")

    inv_n = 1.0 / N_NODES

    with tile.TileContext(nc) as tc:
        with (
            tc.tile_pool(name="const", bufs=1) as constp,
            tc.tile_pool(name="gath", bufs=3) as gathp,
            tc.tile_pool(name="oh", bufs=2) as ohp,
            tc.tile_pool(name="mpsum", bufs=2, space="PSUM") as mpsum,
            tc.tile_pool(name="bpsum", bufs=2, space="PSUM") as bpsum,
            tc.tile_pool(name="tpsum", bufs=2, space="PSUM") as tpsum,
            tc.tile_pool(name="ppsum", bufs=1, space="PSUM") as ppsum,
            tc.tile_pool(name="hn", bufs=4) as hnp,
            tc.tile_pool(name="small", bufs=4) as smallp,
        ):
            # ---- resident constants ----
            idx_sb = constp.tile([P, NT * 8], dt.int16)
            nc.sync.dma_start(out=idx_sb[:], in_=gidx_in[:, :])
            dstloc_sb = constp.tile([P, NT], dt.float32)
            nc.sync.dma_start(out=dstloc_sb[:], in_=dstloc_in[:, :])
            glocal_sb = constp.tile([P, N_CHUNK], dt.float32)
            nc.sync.dma_start(out=glocal_sb[:], in_=glocal_in[:, :])
            w_in_sb = constp.tile([IN_FEAT, HID], dt.float32)
            nc.sync.dma_start(out=w_in_sb[:], in_=w_in_in[:, :])
            conv_sb = [
                constp.tile([HID, HID], dt.float32, tag=f"cw{j}", name=f"cw{j}")
                for j in range(N_CONV)
            ]
            for j in range(N_CONV):
                nc.sync.dma_start(out=conv_sb[j][:], in_=conv_in[j, :, :])
            gamma_sb = constp.tile([P, N_CONV], dt.float32)
            nc.sync.dma_start(out=gamma_sb[:], in_=gamma_in[:, :])
            beta_sb = constp.tile([P, N_CONV], dt.float32)
            nc.sync.dma_start(out=beta_sb[:], in_=beta_in[:, :])
            w1_sb = constp.tile([HID, HID], dt.float32)
            nc.sync.dma_start(out=w1_sb[:], in_=w1_in[:, :])
            b1_sb = constp.tile([P, 1], dt.float32)
            nc.sync.dma_start(out=b1_sb[:], in_=b1_in[:, :])
            w2_sb = constp.tile([HID, 1], dt.float32)
            nc.sync.dma_start(out=w2_sb[:], in_=w2_in[:, :])
            b2_sb = constp.tile([1, 1], dt.float32)
            nc.sync.dma_start(out=b2_sb[:], in_=b2_in[:, :])

            ident = constp.tile([P, P], dt.float32)
            make_identity(nc, ident[:])
            iota_i = constp.tile([P, WIN], dt.int32)
            nc.gpsimd.iota(iota_i[:], pattern=[[1, WIN]], base=0, channel_multiplier=0)
            iota_f = constp.tile([P, WIN], dt.float32)
            nc.vector.tensor_copy(out=iota_f[:], in_=iota_i[:])
            zero_sb = constp.tile([P, 1], dt.float32)
            nc.vector.memset(zero_sb[:], 0.0)

            m_sb = constp.tile([P, SH], dt.float32)       # m then y then h_T
            comb_sb = constp.tile([P, G_SH], dt.float32)  # pooled combination

            # ================= h0 = features @ W_in =================
            for j in range(N_CHUNK):
                fch = hnp.tile([P, IN_FEAT], dt.float32, tag="fch")
                nc.sync.dma_start(out=fch[:], in_=feat_in[j * P : (j + 1) * P, :])
                ftp = tpsum.tile([IN_FEAT, P], dt.float32, tag="tp", name="ftp")
                nc.tensor.transpose(out=ftp[:], in_=fch[:], identity=ident[:])
                ft = hnp.tile([IN_FEAT, P], dt.float32, tag="ft")
                nc.vector.tensor_copy(out=ft[:], in_=ftp[:])
                h0p = tpsum.tile([P, HID], dt.float32, tag="tp", name="h0p")
                nc.tensor.matmul(out=h0p[:], lhsT=ft[:], rhs=w_in_sb[:],
                                 start=True, stop=True)
                h0t = hnp.tile([P, HID], dt.float32, tag="h0t")
                nc.vector.tensor_copy(out=h0t[:], in_=h0p[:])
                nc.sync.dma_start(out=ag_in[j * P : (j + 1) * P, :], in_=h0t[:])
            nc.gpsimd.collective_compute(
                "AllGather", OP.bypass, replica_groups=RG,
                ins=[ag_in[:, :]], outs=[h_tbl[:, :]],
            )

            # ================= 18 message-passing rounds =================
            for r in range(N_ROUNDS):
                j = r % N_CONV
                is_pool = (r % N_CONV) == N_CONV - 1
                pool_i = r // N_CONV
                last = r == N_ROUNDS - 1

                # ---- phase A: m_T = scatter-add of gathered h ----
                tbase = 0  # tile index in the stream
                for k in range(N_BK):
                    # gather calls for this pass
                    ptiles = pass_tiles[k]
                    gts = []  # (tile, offset_in_call) per tile of this pass
                    done = 0
                    while done < ptiles:
                        n_t = min(CALL_TILES, ptiles - done)
                        g = gathp.tile([P, n_t, HID], dt.float32, tag="g")
                        col = (tbase + done) * 8
                        nc.gpsimd.dma_gather(
                            g[:], h_tbl[BUCKET * k : BUCKET * (k + 1), :],
                            idx_sb[:, col : col + n_t * 8],
                            n_t * P, n_t * P, HID, single_packet=False,
                        )
                        for t in range(n_t):
                            gts.append((g, t))
                        done += n_t
                    # onehot + matmul per (b) group
                    ti = 0
                    for b in range(N_WIN):
                        tb = int(T[b, k])
                        ps = mpsum.tile([HID, WIN], dt.float32, tag="mps")
                        t0 = 0
                        while t0 < tb:
                            ng = min(OH_GROUP, tb - t0)
                            oh = ohp.tile([P, ng, WIN], dt.float32, tag="oh")
                            c0 = tbase + ti + t0
                            nc.vector.tensor_tensor(
                                out=oh[:],
                                in0=dstloc_sb[:, c0 : c0 + ng, None].to_broadcast(
                                    [P, ng, WIN]
                                ),
                                in1=iota_f[:, None, :].to_broadcast([P, ng, WIN]),
                                op=OP.is_equal,
                            )
                            for t in range(ng):
                                gt, off = gts[ti + t0 + t]
                                nc.tensor.matmul(
                                    out=ps[:],
                                    lhsT=gt[:, off, :],
                                    rhs=oh[:, t, :],
                                    start=(t0 + t == 0),
                                    stop=(t0 + t == tb - 1),
                                )
                            t0 += ng
                        dst_sl = m_sb[:, WIN * b : WIN * (b + 1)]
                        if k == 0:
                            nc.vector.tensor_copy(out=dst_sl, in_=ps[:])
                        else:
                            nc.vector.tensor_add(out=dst_sl, in0=dst_sl, in1=ps[:])
                        ti += tb
                    tbase += ptiles

                # ---- phase B: y = m @ conv_w[j], BN stats ----
                sum_acc = smallp.tile([P, 1], dt.float32, tag="sum")
                sq_acc = smallp.tile([P, 1], dt.float32, tag="sq")
                for w in range(N_WIN):
                    ps = bpsum.tile([HID, WIN], dt.float32, tag="bps")
                    nc.tensor.matmul(out=ps[:], lhsT=conv_sb[j][:],
                                     rhs=m_sb[:, WIN * w : WIN * (w + 1)],
                                     start=True, stop=True)
                    csum = smallp.tile([P, 1], dt.float32, tag="csum")
                    nc.vector.tensor_reduce(out=csum[:], in_=ps[:], op=OP.add,
                                            axis=mybir.AxisListType.X)
                    sq_scr = hnp.tile([P, WIN], dt.float32, tag="sqscr")
                    csq = smallp.tile([P, 1], dt.float32, tag="csq")
                    nc.scalar.activation(out=sq_scr[:], in_=ps[:], func=AF.Square,
                                         bias=zero_sb[:], accum_out=csq[:])
                    if w == 0:
                        nc.vector.tensor_copy(out=sum_acc[:], in_=csum[:])
                        nc.vector.tensor_copy(out=sq_acc[:], in_=csq[:])
                    else:
                        nc.vector.tensor_add(out=sum_acc[:], in0=sum_acc[:], in1=csum[:])
                        nc.vector.tensor_add(out=sq_acc[:], in0=sq_acc[:], in1=csq[:])
                    nc.vector.tensor_copy(out=m_sb[:, WIN * w : WIN * (w + 1)], in_=ps[:])

                # ---- BN stats allreduce + scale/bias ----
                stat_sb = smallp.tile([P, 2], dt.float32, tag="stat")
                nc.vector.tensor_copy(out=stat_sb[:, 0:1], in_=sum_acc[:])
                nc.vector.tensor_copy(out=stat_sb[:, 1:2], in_=sq_acc[:])
                nc.sync.dma_start(out=ar_in[:, :], in_=stat_sb[:])
                nc.gpsimd.collective_compute(
                    "AllReduce", OP.add, replica_groups=RG,
                    ins=[ar_in[:, :]], outs=[ar_out[:, :]],
                )
                stg = smallp.tile([P, 2], dt.float32, tag="stg")
                nc.sync.dma_start(out=stg[:], in_=ar_out[:, :])
                mean = smallp.tile([P, 1], dt.float32, tag="mean")
                nc.vector.tensor_scalar_mul(out=mean[:], in0=stg[:, 0:1], scalar1=inv_n)
                ex2 = smallp.tile([P, 1], dt.float32, tag="ex2")
                nc.vector.tensor_scalar_mul(out=ex2[:], in0=stg[:, 1:2], scalar1=inv_n)
                var = smallp.tile([P, 1], dt.float32, tag="var")
                nc.vector.tensor_tensor(out=var[:], in0=mean[:], in1=mean[:], op=OP.mult)
                nc.vector.tensor_tensor(out=var[:], in0=ex2[:], in1=var[:], op=OP.subtract)
                sd = smallp.tile([P, 1], dt.float32, tag="sd")
                nc.vector.tensor_scalar_add(out=var[:], in0=var[:], scalar1=BN_EPS)
                nc.scalar.activation(out=sd[:], in_=var[:], func=AF.Sqrt,
                                     bias=zero_sb[:])
                rs = smallp.tile([P, 1], dt.float32, tag="rs")
                nc.vector.reciprocal(out=rs[:], in_=sd[:])
                s_t = smallp.tile([P, 1], dt.float32, tag="s_t")
                nc.vector.tensor_tensor(out=s_t[:], in0=rs[:], in1=gamma_sb[:, j : j + 1], op=OP.mult)
                b_t = smallp.tile([P, 1], dt.float32, tag="b_t")
                nc.vector.tensor_tensor(out=b_t[:], in0=mean[:], in1=s_t[:], op=OP.mult)
                nc.vector.tensor_tensor(out=b_t[:], in0=beta_sb[:, j : j + 1], in1=b_t[:], op=OP.subtract)

                # ---- normalize + relu in place (h_T now in m_sb) ----
                for w in range(N_WIN):
                    sl = m_sb[:, WIN * w : WIN * (w + 1)]
                    nc.scalar.activation(out=sl, in_=sl, func=AF.Relu,
                                         bias=b_t[:], scale=s_t[:])

                # ---- transpose to node-major; pool taps; AG input ----
                if is_pool:
                    pps = ppsum.tile([HID, G_SH], dt.float32, tag="pps")
                for j100 in range(N_CHUNK):
                    tp = tpsum.tile([P, P], dt.float32, tag="tp")
                    nc.tensor.transpose(
                        out=tp[:], in_=m_sb[:, P * j100 : P * (j100 + 1)],
                        identity=ident[:],
                    )
                    hn = hnp.tile([P, HID], dt.float32, tag="hn")
                    nc.vector.tensor_copy(out=hn[:], in_=tp[:])
                    if is_pool:
                        ohg = ohp.tile([P, WIN], dt.float32, tag="ohg")
                        nc.vector.tensor_tensor(
                            out=ohg[:],
                            in0=glocal_sb[:, j100 : j100 + 1].to_broadcast([P, WIN]),
                            in1=iota_f[:],
                            op=OP.is_equal,
                        )
                        nc.tensor.matmul(out=pps[:], lhsT=hn[:], rhs=ohg[:],
                                         start=(j100 == 0), stop=(j100 == N_CHUNK - 1))
                    if not last:
                        nc.sync.dma_start(
                            out=ag_in[P * j100 : P * (j100 + 1), :], in_=hn[:]
                        )
                if is_pool:
                    wgt = float(ngram_w[pool_i])
                    if pool_i == 0:
                        nc.scalar.activation(out=comb_sb[:], in_=pps[:],
                                             func=AF.Copy, scale=wgt)
                    else:
                        ptmp = hnp.tile([P, G_SH], dt.float32, tag="ptmp")
                        nc.scalar.activation(out=ptmp[:], in_=pps[:],
                                             func=AF.Copy, scale=wgt)
                        nc.vector.tensor_add(out=comb_sb[:], in0=comb_sb[:], in1=ptmp[:])
                if not last:
                    nc.gpsimd.collective_compute(
                        "AllGather", OP.bypass, replica_groups=RG,
                        ins=[ag_in[:, :]], outs=[h_tbl[:, :]],
                    )

            # ================= head =================
            z1p = bpsum.tile([HID, G_SH], dt.float32, tag="bps", name="z1p")
            nc.tensor.matmul(out=z1p[:], lhsT=w1_sb[:], rhs=comb_sb[:],
                             start=True, stop=True)
            z1 = hnp.tile([P, G_SH], dt.float32, tag="z1")
            nc.scalar.activation(out=z1[:], in_=z1p[:], func=AF.Lrelu,
                                 bias=b1_sb[:], alpha=0.01)
            z2p = bpsum.tile([1, G_SH], dt.float32, tag="bps", name="z2p")
            nc.tensor.matmul(out=z2p[:], lhsT=w2_sb[:], rhs=z1[:],
                             start=True, stop=True)
            z2 = smallp.tile([1, G_SH], dt.float32, tag="z2")
            nc.scalar.activation(out=z2[:], in_=z2p[:], func=AF.Sigmoid,
                                 bias=b2_sb[:1, :])
            nc.sync.dma_start(out=out_t[:, :], in_=z2[:])

    nc.compile()
    return nc


_CACHE = {}


def kernel(features, W_in, conv_w, bn_gamma, bn_beta, ngram_weights,
           W1, b1, W2, b2, src, dst, graph_ids):
    from concourse import bass_utils

    prep = _prep(features, src, dst, graph_ids)

    ngw = np.asarray(ngram_weights, dtype=np.float64)
    e = np.exp(ngw - ngw.max())
    ngram_w = (e / e.sum()).astype(np.float64)

    key = (prep["NT"], tuple(prep["pass_tiles"]), tuple(np.asarray(ngram_w).tolist()))
    if key not in _CACHE:
        _CACHE[key] = _build(prep["T"], prep["NT"], prep["pass_tiles"], ngram_w)
    nc = _CACHE[key]

    gamma_t = np.asarray(bn_gamma, dtype=np.float32).T.copy()  # [128, 3]
    beta_t = np.asarray(bn_beta, dtype=np.float32).T.copy()
    b1_t = np.asarray(b1, dtype=np.float32).reshape(P, 1)
    b2_t = np.asarray(b2, dtype=np.float32).reshape(1, 1)

    in_maps = []
    for c in range(N_CORES):
        in_maps.append({
            "feat": prep["feat"][c],
            "gidx": prep["gidx"][c],
            "dstloc": prep["dstloc"][c],
            "glocal": prep["glocal"][c],
            "w_in": np.asarray(W_in, dtype=np.float32),
            "conv_w": np.asarray(conv_w, dtype=np.float32),
            "gamma_t": gamma_t,
            "beta_t": beta_t,
            "w1": np.asarray(W1, dtype=np.float32),
            "b1_t": b1_t,
            "w2": np.asarray(W2, dtype=np.float32),
            "b2_t": b2_t,
        })

    trace = bool(int(os.environ.get("KTRACE", "0")))
    if trace:
        try:
            import sys, types
            if "antenv.axon_hooks" not in sys.modules:
                mod = types.ModuleType("antenv.axon_hooks")
                _h = [None]
                mod.set_axon_ntff_profile_hook = lambda h: _h.__setitem__(0, h)
                mod.get_axon_ntff_profile_hook = lambda: _h[0]
                sys.modules["antenv.axon_hooks"] = mod
                import antenv
                antenv.axon_hooks = mod
            from antenv.axon_hooks import get_axon_ntff_profile_hook, set_axon_ntff_profile_hook
            if get_axon_ntff_profile_hook() is None:
                from trn_agent_boot.trn_boot import _ntff_profile_via_ctypes
                set_axon_ntff_profile_hook(
                    _ntff_profile_via_ctypes("/opt/axon/libaxon_pjrt.so"))
        except Exception:
            trace = False
    res = bass_utils.run_bass_kernel_spmd(nc, in_maps, core_ids=list(range(N_CORES)),
                                          trace=trace)
    if trace and res.exec_time_ns is not None:
        print(f"HW exec time: {res.exec_time_ns} ns")
    out = np.concatenate([res.results[c]["out"][0] for c in range(N_CORES)])
    return out.reshape(N_GRAPHS, 1).astype(np.float32)
